# revision 34
# baseline (speedup 1.0000x reference)
"""Trainium2 Bass kernel for the GNN message-passing model.

Math (reference):
    base[b,s,t,j] = x[b,s,t,j]            (j<4)
    extra[b,s,t,c] = x[b,s,t,4+c]
    h_pre[b,c,s,h] = sum_{t,j} base[b,s,t,j]*mW1[5t+j,h]
                   + sum_t extra[b,s,t,c]*mW1[5t+4,h] + mb1[h]
    msg_sum[b,c,:] = sum_s relu(h_pre[b,c,s,:]) @ mW2 + N*mb2
    out = MLP(concat(msg_sum, x[:,:,-1,:4]))

Decomposition used here:
  * A[b,s,h] = base-part + mb1 is shared across all columns c -> precomputed
    on host (21 MFLOP of ~26 GFLOP total) and folded into the matmul as an
    extra contraction row against a ones-row in the rhs.
  * Per (b,s) pair the device does ONE matmul [K=11, M=128h, N=512c]
    producing h_pre for all columns, then relu, then accumulates over s.
  * sum_s(relu(h) @ mW2) == (sum_s relu(h)) @ mW2, and further the mW2
    matmul itself can BE the s-accumulator: PSUM accumulation of
    mW2.T @ relu_tile over s (bf16 relu tiles; the tiny mW2 in bf16).
    Alternating pairs instead use a fused DVE scalar_tensor_tensor
    (H = max(psum,0) + H, fp32) to balance ACT/DVE/PE load.
  * The big per-pair matmul runs in float32r (full fp32 bits, PE consumes
    them at 1 cycle/row instead of fp32's 4) -> ~2.7x end-to-end.
  * Sharding: data-parallel over the source axis s (512 -> 64 per core);
    each core produces a partial msg[b,32,c] for all columns; host sums the
    8 partials and runs the tiny update MLP (0.15% of FLOPs) in numpy.
"""

import os
import numpy as np

import concourse.bass as bass
import concourse.mybir as mybir
from concourse.tile import TileContext
from concourse.bass_utils import run_bass_kernel_spmd

B, N, T, F = 4, 512, 10, 516
HID, MSG = 128, 32
NCORES = 8
SLOC = N // NCORES          # source rows per core
K1 = T + 1                  # 10 extra-feature rows + 1 ones-row (bias fold)
F32 = mybir.dt.float32
F32R = mybir.dt.float32r
BF16 = mybir.dt.bfloat16

# fraction of (b,s) pairs whose relu runs on the scalar engine (ACT) with the
# accumulate done on the tensor engine; the rest use the fused DVE op.
# pair-type pattern: "A" = ACT relu + PE (mW2) accumulate,
# "B" = fused DVE accumulate (hacc = max(psum,0) + hacc).
PATTERN = ["A", "B"] * 8
MM1_F32R = True     # use float32r for the big per-pair matmul
ACC_BF16 = True     # bf16 relu output + bf16 mW2 accumulate matmul

_prog = None
last_results = None

# Tile emits semaphore waits for same-engine WAW/RAW deps (e.g. an ACT op
# waiting on the ACT sem for a pool buffer recycled from an older ACT write).
# Compute engines execute strictly in order, so these waits are redundant --
# and they overflow the 1-slot sync-wait budget of several ISA structs
# (ACTIVATE, TensorScalarPtr). Strip them post-scheduling.
_STRIP_TYPES = {
    "InstActivation", "InstTensorScalarPtr", "InstTensorTensor",
    "InstTensorCopy", "InstTensorReduce", "InstMatmult", "InstMemSet",
}
_ENG2SEM = None


def _strip_self_waits(nc):
    global _ENG2SEM
    if _ENG2SEM is None:
        _ENG2SEM = {
            mybir.EngineType.PE: "PE_",
            mybir.EngineType.Activation: "Activation_",
            mybir.EngineType.DVE: "DVE_",
            mybir.EngineType.Pool: "Pool_",
        }
    for fn in nc.m.functions:
        for blk in fn.blocks:
            for inst in blk.instructions:
                if type(inst).__name__ not in _STRIP_TYPES:
                    continue
                si = inst.sync_info
                if si is None or not si.on_wait:
                    continue
                pre = _ENG2SEM.get(inst.engine)
                if pre is None:
                    continue
                kept = [w for w in si.on_wait if not (w.ant_name or "").startswith(pre)]
                if len(kept) != len(si.on_wait):
                    si.on_wait = kept
    # Chunk-load DMAs: the WAR wait on the engine that read the recycled
    # buffer transitively dominates the WAW wait on the DMA that previously
    # filled it (that engine's reads each waited on that DMA themselves).
    eng_sems = ("PE_", "Activation_", "DVE_", "Pool_")
    for fn in nc.m.functions:
        for blk in fn.blocks:
            for inst in blk.instructions:
                if type(inst).__name__ != "InstDMACopy":
                    continue
                si = inst.sync_info
                if si is None or not si.on_wait:
                    continue
                has_eng = any((w.ant_name or "").startswith(eng_sems) for w in si.on_wait)
                if not has_eng:
                    continue
                kept = [
                    w for w in si.on_wait
                    if not (w.ant_name or "").startswith(("DMAHW", "DMASW"))
                ]
                if len(kept) != len(si.on_wait):
                    si.on_wait = kept
    # Kernel-tail Drain: waits on every DMA queue overflow the CTRL struct's
    # wait budget. Input-DMA waits are dominated by the engine waits (each
    # load was read by a compute engine before the drain); only the queues
    # carrying the output DMAs must be waited on directly.
    out_sems = set()
    for fn in nc.m.functions:
        for blk in fn.blocks:
            for inst in blk.instructions:
                if type(inst).__name__ != "InstDMACopy":
                    continue
                outs = getattr(inst, "outs", None) or []
                to_dram = any("msg_out" in (getattr(o, "memref", "") or "")
                              for o in outs)
                si = inst.sync_info
                if to_dram and si and si.on_update:
                    for u in si.on_update:
                        out_sems.add(u.ant_name)
    drain_split = 0
    for fn in nc.m.functions:
        for blk in fn.blocks:
            for ii in range(len(blk.instructions)):
                inst = blk.instructions[ii]
                if type(inst).__name__ != "InstDrain":
                    continue
                si = inst.sync_info
                if si is None or not si.on_wait or len(si.on_wait) <= 1:
                    continue
                waits = [
                    w for w in si.on_wait
                    if not (w.ant_name or "").startswith(("DMAHW", "DMASW"))
                    or w.ant_name in out_sems
                ]
                # split into a chain of drains with one wait each (the SP
                # CTRL struct has a single sync-wait slot)
                pre = []
                while len(waits) > 1:
                    chunk, waits = waits[:1], waits[1:]
                    d = mybir.InstDrain(
                        name=f"{inst.name}_split{drain_split}", ins=[], outs=[],
                        sync_info=mybir.SyncInfo(on_wait=chunk, on_update=[]),
                    )
                    d.engine = inst.engine
                    drain_split += 1
                    pre.append(d)
                si.on_wait = waits
                for d in reversed(pre):
                    blk.instructions.insert(ii, d)
                break


def _build_program():
    nc = bass.Bass(trn_type="TRN2")
    # packed input: per (b, s) an [K1, N+HID] block -- first N cols are the
    # matmul rhs (extra features + ones row), last HID cols the per-pair lhsT
    # (W1x rows + folded bias row). One tensor -> one DMA sem per chunk.
    extdt = F32R if MM1_F32R else F32
    ext = nc.dram_tensor("ext", [B, SLOC, K1, N + HID], extdt, kind="ExternalInput")
    w2 = nc.dram_tensor("w2", [HID, MSG], F32, kind="ExternalInput")
    w2b = nc.dram_tensor("w2b", [HID, MSG], BF16, kind="ExternalInput")
    msg_out = nc.dram_tensor("msg_out", [B, MSG, N], F32, kind="ExternalOutput")

    CH = 16  # source rows per DMA chunk
    with TileContext(nc) as tc:
        with (
            tc.tile_pool(name="const", bufs=1) as constp,
            tc.tile_pool(name="big", bufs=2) as bigp,
            tc.tile_pool(name="relua", bufs=4) as rap,   # ACT-relu'd, read by PE
            tc.tile_pool(name="hacc", bufs=2) as hp,
            tc.tile_pool(name="out", bufs=4) as outp,
            tc.tile_pool(name="ps", bufs=3, space="PSUM") as pp,
            tc.tile_pool(name="pwarm", bufs=1, space="PSUM") as pwp,
            tc.tile_pool(name="pacc", bufs=2, space="PSUM") as pa,
        ):
            w2t = constp.tile([HID, MSG], F32)
            nc.sync.dma_start(w2t[:], w2[:])
            w2bt = constp.tile([HID, MSG], BF16, tag="w2bt")
            nc.sync.dma_start(w2bt[:], w2b[:])
            # warmup touch of w2t on PE so later macc matmuls don't need a
            # DMA wait on top of their relu-tile wait
            warm = pwp.tile([MSG, 1], F32, tag="warm")
            nc.tensor.matmul(warm[:], w2t[:], w2t[:, :1], start=True, stop=True)
            warm2 = pwp.tile([MSG, 1], F32, tag="warm")
            nc.tensor.matmul(warm2[:], w2bt[:], w2bt[:, :1], start=True, stop=True)

            for b in range(B):
                hacc = None
                macc = pa.tile([MSG, N], F32, tag="macc")
                nmm = 0
                hacc_init = False
                for g in range(SLOC // CH):
                    big_t = bigp.tile([K1, CH, N + HID], extdt, tag="big")
                    nc.sync.dma_start(
                        big_t[:],
                        ext[b, g * CH:(g + 1) * CH].rearrange("s k c -> k s c"),
                    )
                    # tiny PE touch of the fresh chunk: absorbs the DMA wait
                    # so the first real matmul only waits on the PSUM recycle
                    wt = pwp.tile([MSG, 1], F32, tag="warm")
                    nc.tensor.matmul(
                        wt[:, :1],
                        big_t[:, 0, :MSG].bitcast(F32),
                        big_t[:, 0, :1].bitcast(F32),
                        start=True, stop=True,
                    )
                    for si in range(CH):
                        s = g * CH + si
                        p = b * SLOC + s
                        ty = PATTERN[p % len(PATTERN)]
                        ps = pp.tile([HID, N], F32, tag="ps")
                        nc.tensor.matmul(
                            ps[:], big_t[:, si, N:N + HID], big_t[:, si, :N],
                            start=True, stop=True,
                        )
                        if ty == "A":
                            r = rap.tile([HID, N], BF16 if ACC_BF16 else F32, tag="relua")
                            nc.scalar.activation(
                                r[:], ps[:], mybir.ActivationFunctionType.Relu
                            )
                            nc.tensor.matmul(
                                macc[:], w2bt[:] if ACC_BF16 else w2t[:], r[:],
                                start=(nmm == 0), stop=False,
                                skip_group_check=True,
                            )
                            nmm += 1
                        else:  # "B": fused DVE relu+accumulate from PSUM
                            if not hacc_init:
                                hacc = hp.tile([HID, N], F32, tag="hacc")
                                nc.vector.tensor_scalar(
                                    hacc[:], ps[:], 0.0, None,
                                    op0=mybir.AluOpType.max,
                                )
                                hacc_init = True
                            else:
                                nc.vector.scalar_tensor_tensor(
                                    hacc[:], ps[:], 0.0, hacc[:],
                                    op0=mybir.AluOpType.max,
                                    op1=mybir.AluOpType.add,
                                )
                # fold the DVE-accumulated part through mW2 as well
                if hacc_init:
                    nc.tensor.matmul(
                        macc[:], w2t[:], hacc[:],
                        start=(nmm == 0), stop=True,
                        skip_group_check=True,
                    )
                ot = outp.tile([MSG, N], F32, tag="ot")
                nc.scalar.copy(ot[:], macc[:])
                nc.sync.dma_start(msg_out[b], ot[:])
    _strip_self_waits(nc)
    return nc


def _get_prog():
    global _prog
    if _prog is None:
        _prog = _build_program()
    return _prog


def kernel(x, mW1, mb1, mW2, mb2, iW1, ib1, iW2, ib2):
    global last_results
    x = np.ascontiguousarray(np.asarray(x, dtype=np.float32))
    mW1 = np.asarray(mW1, dtype=np.float32)
    mb1 = np.asarray(mb1, dtype=np.float32)
    mW2 = np.ascontiguousarray(np.asarray(mW2, dtype=np.float32))
    mb2 = np.asarray(mb2, dtype=np.float32)

    # host prep: A[b,s,h] = base_flat @ W1b + mb1 (tiny), weight slices
    base = x[:, :, :, :4]                                  # [B,N,T,4]
    base_flat = base.reshape(B, N, T * 4)
    W1b = mW1.reshape(T, 5, HID)[:, :4, :].reshape(T * 4, HID)
    W1x = np.ascontiguousarray(mW1.reshape(T, 5, HID)[:, 4, :])   # [T,HID]
    A = base_flat @ W1b + mb1                              # [B,N,HID]
    import ml_dtypes
    mW2b = mW2.astype(ml_dtypes.bfloat16)

    # per-core inputs
    in_maps = []
    for k in range(NCORES):
        sl = slice(k * SLOC, (k + 1) * SLOC)
        ext_k = np.empty((B, SLOC, K1, N + HID), dtype=np.float32)
        ext_k[:, :, :T, :N] = x[:, sl, :, 4:4 + N]
        ext_k[:, :, T, :N] = 1.0
        ext_k[:, :, :T, N:] = W1x[None, None, :, :]
        ext_k[:, :, T, N:] = A[:, sl, :]
        in_maps.append({
            "ext": np.ascontiguousarray(ext_k),
            "w2": mW2,
            "w2b": mW2b,
        })

    nc = _get_prog()
    trace = bool(int(os.environ.get("KERNEL_TRACE", "0")))
    try:
        res = run_bass_kernel_spmd(
            nc, in_maps, core_ids=list(range(NCORES)), trace=trace,
        )
    except ModuleNotFoundError:
        # axon NTFF profiling hook unavailable -> rerun without trace
        res = run_bass_kernel_spmd(
            nc, in_maps, core_ids=list(range(NCORES)), trace=False,
        )
    last_results = res

    msg_part = np.zeros((B, MSG, N), dtype=np.float32)
    for r in res.results:
        msg_part += r["msg_out"]

    msg_sum = np.transpose(msg_part, (0, 2, 1)) + N * mb2  # [B,N,MSG]
    node_feat = x[:, :, -1, :4]
    mi = np.concatenate([msg_sum, node_feat], axis=-1)     # [B,N,MSG+4]
    h2 = np.maximum(mi @ np.asarray(iW1, dtype=np.float32)
                    + np.asarray(ib1, dtype=np.float32), 0.0)
    out = h2 @ np.asarray(iW2, dtype=np.float32) + np.asarray(ib2, dtype=np.float32)
    return out.astype(np.float32)
